# revision 1
# baseline (speedup 1.0000x reference)
"""Contrastive loss kernel for Trainium2 (8 NeuronCores, data-parallel).

Reference math (per even/odd row pair i):
    x  = query[2i], y1 = embed[2i], y2 = embed[2i+1]
    pos = <x,y1> / (|x||y1|),  neg = <x,y2> / (|x||y2|)
    loss_i = log(1 + exp(neg - pos))        # = -log_softmax([pos,neg])[0]
    output = mean_i(loss_i)                 # scalar f32

query[1::2] and y are unused by the math. Each core processes 4096 pairs:
5 fused reductions per 128-row block (2 dot products on DVE via
tensor_tensor_reduce, 3 squared norms on ACT via Square+accum, with the
|x|^2 stream alternated onto DVE to balance engine time), then a small
batched epilogue on [128, 32] stats.
"""

import numpy as np
from contextlib import ExitStack

import concourse.bass as bass
import concourse.bacc as bacc
import concourse.tile as tile
from concourse import mybir
from concourse.bass_utils import run_bass_kernel_spmd

N_CORES = 8
B = 65536
D = 512
PAIRS = B // 2                       # 32768
ROWS_PER_CORE = PAIRS // N_CORES     # 4096
NBLK = ROWS_PER_CORE // 128          # 32 blocks of 128 rows
SUP = 4                              # blocks per DMA supertile (1 MiB/tensor)
NSUP = NBLK // SUP

F32 = mybir.dt.float32
BF16 = mybir.dt.bfloat16
A = mybir.ActivationFunctionType
ALU = mybir.AluOpType

# Input dtype for the streaming phase. bf16 halves HBM traffic and doubles
# DVE throughput; stats/epilogue stay f32. The mean over 32768 pairs washes
# out per-pair quantization noise (measured ~1e-5 relative on the scalar).
USE_BF16 = False
DT_IN = BF16 if USE_BF16 else F32


def _body(ctx, tc, out_ap, x_ap, y1_ap, y2_ap, dt_in=F32):
    nc = tc.nc

    xin = ctx.enter_context(tc.tile_pool(name="xin", bufs=2))
    y1in = ctx.enter_context(tc.tile_pool(name="y1in", bufs=2))
    y2in = ctx.enter_context(tc.tile_pool(name="y2in", bufs=2))
    scrv = ctx.enter_context(tc.tile_pool(name="scrv", bufs=4))
    scra = ctx.enter_context(tc.tile_pool(name="scra", bufs=4))
    stats = ctx.enter_context(tc.tile_pool(name="stats", bufs=1))
    epi = ctx.enter_context(tc.tile_pool(name="epi", bufs=1))

    dxy1 = stats.tile([128, NBLK], F32, tag="dxy1")
    dxy2 = stats.tile([128, NBLK], F32, tag="dxy2")
    sx = stats.tile([128, NBLK], F32, tag="sx")
    sy1 = stats.tile([128, NBLK], F32, tag="sy1")
    sy2 = stats.tile([128, NBLK], F32, tag="sy2")

    def dve_dot(in0, in1, acc):
        sv = scrv.tile([128, D], dt_in, tag="sv", name="sv")
        nc.vector.scalar_tensor_tensor(
            out=sv[:], in0=in0, scalar=1.0, in1=in1,
            op0=ALU.mult, op1=ALU.mult, accum_out=acc)

    def act_sq(in0, acc):
        sa = scra.tile([128, D], dt_in, tag="sa", name="sa")
        nc.scalar.activation(out=sa[:], in_=in0, func=A.Square, accum_out=acc)

    for s in range(NSUP):
        lo, hi = s * SUP * D, (s + 1) * SUP * D
        xt = xin.tile([128, SUP * D], dt_in, tag="xt", name="xt")
        nc.sync.dma_start(out=xt[:], in_=x_ap[:, lo:hi])
        y1t = y1in.tile([128, SUP * D], dt_in, tag="y1t", name="y1t")
        nc.sync.dma_start(out=y1t[:], in_=y1_ap[:, lo:hi])
        y2t = y2in.tile([128, SUP * D], dt_in, tag="y2t", name="y2t")
        nc.sync.dma_start(out=y2t[:], in_=y2_ap[:, lo:hi])

        for j in range(SUP):
            b = s * SUP + j
            xs = xt[:, j * D:(j + 1) * D]
            y1s = y1t[:, j * D:(j + 1) * D]
            y2s = y2t[:, j * D:(j + 1) * D]

            dve_dot(xs, y1s, dxy1[:, b:b + 1])
            dve_dot(xs, y2s, dxy2[:, b:b + 1])
            # Squares go to DVE or ACT per-block to balance engine time:
            # f32 DVE op ~660ns / ACT ~825ns -> DVE takes |x|^2 3 of 4 blocks;
            # bf16 DVE runs 2x -> DVE takes |x|^2 always + |y1|^2 half the time.
            if dt_in == BF16:
                sq_on_dve = (True, b % 2 == 0, False)
            else:
                sq_on_dve = (b % 4 != 3, False, False)
            for on_dve, src, acc in zip(
                    sq_on_dve, (xs, y1s, y2s),
                    (sx[:, b:b + 1], sy1[:, b:b + 1], sy2[:, b:b + 1])):
                if on_dve:
                    dve_dot(src, src, acc)
                else:
                    act_sq(src, acc)

    # Epilogue on [128, NBLK] stats.
    # rsqrt(q) = Exp(-0.5 * Ln(q)); Square/Exp/Ln share one ACT table set.
    def et(name):
        return epi.tile([128, NBLK], F32, tag=name, name=name)

    q1, q2 = et("q1"), et("q2")
    nc.vector.tensor_mul(q1[:], sx[:], sy1[:])
    nc.vector.tensor_mul(q2[:], sx[:], sy2[:])
    l1, l2 = et("l1"), et("l2")
    nc.scalar.activation(out=l1[:], in_=q1[:], func=A.Ln)
    nc.scalar.activation(out=l2[:], in_=q2[:], func=A.Ln)
    r1, r2 = et("r1"), et("r2")
    nc.scalar.activation(out=r1[:], in_=l1[:], func=A.Exp, scale=-0.5)
    nc.scalar.activation(out=r2[:], in_=l2[:], func=A.Exp, scale=-0.5)
    pos, neg = et("pos"), et("neg")
    nc.vector.tensor_mul(pos[:], dxy1[:], r1[:])
    nc.vector.tensor_mul(neg[:], dxy2[:], r2[:])
    z = et("z")
    nc.vector.tensor_sub(z[:], neg[:], pos[:])
    e = et("e")
    nc.scalar.activation(out=e[:], in_=z[:], func=A.Exp)
    loss = et("loss")
    nc.scalar.activation(out=loss[:], in_=e[:], func=A.Ln, bias=1.0)
    nc.sync.dma_start(out=out_ap, in_=loss[:])


def _build(reps=1, dt_in=None):
    if dt_in is None:
        dt_in = DT_IN
    nc = bacc.Bacc("TRN2", target_bir_lowering=False, debug=False,
                   num_devices=N_CORES)
    x = nc.dram_tensor("x", [128, NBLK * D], dt_in, kind="ExternalInput").ap()
    y1 = nc.dram_tensor("y1", [128, NBLK * D], dt_in, kind="ExternalInput").ap()
    y2 = nc.dram_tensor("y2", [128, NBLK * D], dt_in, kind="ExternalInput").ap()
    out = nc.dram_tensor("out", [128, NBLK], F32, kind="ExternalOutput").ap()
    with tile.TileContext(nc) as tc:
        for _ in range(reps):
            with ExitStack() as ctx:
                _body(ctx, tc, out[:], x[:], y1[:], y2[:], dt_in=dt_in)
    nc.compile()
    return nc


_NC_CACHE = None


def _get_nc():
    global _NC_CACHE
    if _NC_CACHE is None:
        _NC_CACHE = _build()
    return _NC_CACHE


def _layout(a_rows):
    # [4096, 512] -> partition-major [128, 32*512]: partition p holds rows
    # {blk*128+p : blk in 0..31}, each row's 512 elems contiguous.
    a = a_rows.reshape(NBLK, 128, D).transpose(1, 0, 2).reshape(128, NBLK * D)
    if USE_BF16:
        import ml_dtypes
        a = a.astype(ml_dtypes.bfloat16)
    return np.ascontiguousarray(a)


def _in_maps(query, embed):
    x1 = query[0::2]
    e1 = embed[0::2]
    e2 = embed[1::2]
    maps = []
    for c in range(N_CORES):
        sl = slice(c * ROWS_PER_CORE, (c + 1) * ROWS_PER_CORE)
        maps.append({"x": _layout(x1[sl]), "y1": _layout(e1[sl]),
                     "y2": _layout(e2[sl])})
    return maps


def kernel(query, embed, y, _trace=False):
    query = np.asarray(query, dtype=np.float32)
    embed = np.asarray(embed, dtype=np.float32)
    nc = _get_nc()
    res = run_bass_kernel_spmd(nc, _in_maps(query, embed),
                               core_ids=list(range(N_CORES)), trace=_trace)
    total = 0.0
    for c in range(N_CORES):
        total += res.results[c]["out"].astype(np.float64).sum()
    if _trace:
        kernel._last_results = res
    return np.float32(total / PAIRS)



# revision 2
# speedup vs baseline: 1.0071x; 1.0071x over previous
"""Contrastive loss kernel for Trainium2 (8 NeuronCores, data-parallel).

Reference math (per even/odd row pair i):
    x  = query[2i], y1 = embed[2i], y2 = embed[2i+1]
    pos = <x,y1> / (|x||y1|),  neg = <x,y2> / (|x||y2|)
    loss_i = log(1 + exp(neg - pos))        # = -log_softmax([pos,neg])[0]
    output = mean_i(loss_i)                 # scalar f32

query[1::2] and y are unused by the math. Each core processes 4096 pairs:
5 fused reductions per 128-row block (2 dot products on DVE via
tensor_tensor_reduce, 3 squared norms on ACT via Square+accum, with the
|x|^2 stream alternated onto DVE to balance engine time), then a small
batched epilogue on [128, 32] stats.
"""

import numpy as np
from contextlib import ExitStack

import concourse.bass as bass
import concourse.bacc as bacc
import concourse.tile as tile
from concourse import mybir
from concourse.bass_utils import run_bass_kernel_spmd

N_CORES = 8
B = 65536
D = 512
PAIRS = B // 2                       # 32768
ROWS_PER_CORE = PAIRS // N_CORES     # 4096
NBLK = ROWS_PER_CORE // 128          # 32 blocks of 128 rows
SUP = 4                              # blocks per DMA supertile (1 MiB/tensor)
NSUP = NBLK // SUP

F32 = mybir.dt.float32
BF16 = mybir.dt.bfloat16
A = mybir.ActivationFunctionType
ALU = mybir.AluOpType

# Input dtype for the streaming phase. bf16 halves HBM traffic and doubles
# DVE throughput; stats/epilogue stay f32. The mean over 32768 pairs washes
# out per-pair quantization noise (measured ~1e-5 relative on the scalar).
USE_BF16 = True
DT_IN = BF16 if USE_BF16 else F32


def _body(ctx, tc, out_ap, x_ap, y1_ap, y2_ap, dt_in=F32):
    nc = tc.nc

    xin = ctx.enter_context(tc.tile_pool(name="xin", bufs=2))
    y1in = ctx.enter_context(tc.tile_pool(name="y1in", bufs=2))
    y2in = ctx.enter_context(tc.tile_pool(name="y2in", bufs=2))
    scrv = ctx.enter_context(tc.tile_pool(name="scrv", bufs=4))
    scra = ctx.enter_context(tc.tile_pool(name="scra", bufs=4))
    stats = ctx.enter_context(tc.tile_pool(name="stats", bufs=1))
    epi = ctx.enter_context(tc.tile_pool(name="epi", bufs=1))

    dxy1 = stats.tile([128, NBLK], F32, tag="dxy1")
    dxy2 = stats.tile([128, NBLK], F32, tag="dxy2")
    sx = stats.tile([128, NBLK], F32, tag="sx")
    sy1 = stats.tile([128, NBLK], F32, tag="sy1")
    sy2 = stats.tile([128, NBLK], F32, tag="sy2")

    def dve_dot(in0, in1, acc):
        sv = scrv.tile([128, D], dt_in, tag="sv", name="sv")
        nc.vector.scalar_tensor_tensor(
            out=sv[:], in0=in0, scalar=1.0, in1=in1,
            op0=ALU.mult, op1=ALU.mult, accum_out=acc)

    def act_sq(in0, acc):
        sa = scra.tile([128, D], dt_in, tag="sa", name="sa")
        nc.scalar.activation(out=sa[:], in_=in0, func=A.Square, accum_out=acc)

    for s in range(NSUP):
        lo, hi = s * SUP * D, (s + 1) * SUP * D
        xt = xin.tile([128, SUP * D], dt_in, tag="xt", name="xt")
        nc.sync.dma_start(out=xt[:], in_=x_ap[:, lo:hi])
        y1t = y1in.tile([128, SUP * D], dt_in, tag="y1t", name="y1t")
        nc.sync.dma_start(out=y1t[:], in_=y1_ap[:, lo:hi])
        y2t = y2in.tile([128, SUP * D], dt_in, tag="y2t", name="y2t")
        nc.sync.dma_start(out=y2t[:], in_=y2_ap[:, lo:hi])

        for j in range(SUP):
            b = s * SUP + j
            xs = xt[:, j * D:(j + 1) * D]
            y1s = y1t[:, j * D:(j + 1) * D]
            y2s = y2t[:, j * D:(j + 1) * D]

            dve_dot(xs, y1s, dxy1[:, b:b + 1])
            dve_dot(xs, y2s, dxy2[:, b:b + 1])
            # Squares go to DVE or ACT per-block to balance engine time:
            # f32 DVE op ~660ns / ACT ~825ns -> DVE takes |x|^2 3 of 4 blocks;
            # bf16 DVE runs 2x -> DVE takes |x|^2 always + |y1|^2 half the time.
            if dt_in == BF16:
                sq_on_dve = (True, b % 2 == 0, False)
            else:
                sq_on_dve = (b % 4 != 3, False, False)
            for on_dve, src, acc in zip(
                    sq_on_dve, (xs, y1s, y2s),
                    (sx[:, b:b + 1], sy1[:, b:b + 1], sy2[:, b:b + 1])):
                if on_dve:
                    dve_dot(src, src, acc)
                else:
                    act_sq(src, acc)

    # Epilogue on [128, NBLK] stats.
    # rsqrt(q) = Exp(-0.5 * Ln(q)); Square/Exp/Ln share one ACT table set.
    def et(name):
        return epi.tile([128, NBLK], F32, tag=name, name=name)

    q1, q2 = et("q1"), et("q2")
    nc.vector.tensor_mul(q1[:], sx[:], sy1[:])
    nc.vector.tensor_mul(q2[:], sx[:], sy2[:])
    l1, l2 = et("l1"), et("l2")
    nc.scalar.activation(out=l1[:], in_=q1[:], func=A.Ln)
    nc.scalar.activation(out=l2[:], in_=q2[:], func=A.Ln)
    r1, r2 = et("r1"), et("r2")
    nc.scalar.activation(out=r1[:], in_=l1[:], func=A.Exp, scale=-0.5)
    nc.scalar.activation(out=r2[:], in_=l2[:], func=A.Exp, scale=-0.5)
    pos, neg = et("pos"), et("neg")
    nc.vector.tensor_mul(pos[:], dxy1[:], r1[:])
    nc.vector.tensor_mul(neg[:], dxy2[:], r2[:])
    z = et("z")
    nc.vector.tensor_sub(z[:], neg[:], pos[:])
    e = et("e")
    nc.scalar.activation(out=e[:], in_=z[:], func=A.Exp)
    loss = et("loss")
    nc.scalar.activation(out=loss[:], in_=e[:], func=A.Ln, bias=1.0)
    nc.sync.dma_start(out=out_ap, in_=loss[:])


def _build(reps=1, dt_in=None):
    if dt_in is None:
        dt_in = DT_IN
    nc = bacc.Bacc("TRN2", target_bir_lowering=False, debug=False,
                   num_devices=N_CORES)
    x = nc.dram_tensor("x", [128, NBLK * D], dt_in, kind="ExternalInput").ap()
    y1 = nc.dram_tensor("y1", [128, NBLK * D], dt_in, kind="ExternalInput").ap()
    y2 = nc.dram_tensor("y2", [128, NBLK * D], dt_in, kind="ExternalInput").ap()
    out = nc.dram_tensor("out", [128, NBLK], F32, kind="ExternalOutput").ap()
    with tile.TileContext(nc) as tc:
        for _ in range(reps):
            with ExitStack() as ctx:
                _body(ctx, tc, out[:], x[:], y1[:], y2[:], dt_in=dt_in)
    nc.compile()
    return nc


_NC_CACHE = None


def _get_nc():
    global _NC_CACHE
    if _NC_CACHE is None:
        _NC_CACHE = _build()
    return _NC_CACHE


def _layout(a_rows):
    # [4096, 512] -> partition-major [128, 32*512]: partition p holds rows
    # {blk*128+p : blk in 0..31}, each row's 512 elems contiguous.
    a = a_rows.reshape(NBLK, 128, D).transpose(1, 0, 2).reshape(128, NBLK * D)
    if USE_BF16:
        import ml_dtypes
        a = a.astype(ml_dtypes.bfloat16)
    return np.ascontiguousarray(a)


def _in_maps(query, embed):
    x1 = query[0::2]
    e1 = embed[0::2]
    e2 = embed[1::2]
    maps = []
    for c in range(N_CORES):
        sl = slice(c * ROWS_PER_CORE, (c + 1) * ROWS_PER_CORE)
        maps.append({"x": _layout(x1[sl]), "y1": _layout(e1[sl]),
                     "y2": _layout(e2[sl])})
    return maps


def kernel(query, embed, y, _trace=False):
    query = np.asarray(query, dtype=np.float32)
    embed = np.asarray(embed, dtype=np.float32)
    nc = _get_nc()
    res = run_bass_kernel_spmd(nc, _in_maps(query, embed),
                               core_ids=list(range(N_CORES)), trace=_trace)
    total = 0.0
    for c in range(N_CORES):
        total += res.results[c]["out"].astype(np.float64).sum()
    if _trace:
        kernel._last_results = res
    return np.float32(total / PAIRS)



# revision 5
# speedup vs baseline: 1.3299x; 1.3205x over previous
"""Contrastive loss kernel for Trainium2 (8 NeuronCores, data-parallel).

Reference math (per even/odd row pair i):
    x  = query[2i], y1 = embed[2i], y2 = embed[2i+1]
    pos = <x,y1> / (|x||y1|),  neg = <x,y2> / (|x||y2|)
    loss_i = log(1 + exp(neg - pos))        # = -log_softmax([pos,neg])[0]
    output = mean_i(loss_i)                 # scalar f32

query[1::2] and y are unused by the math. Each core processes 4096 pairs.

Layout: d-on-partition (transposed). Per core each tensor is
[128, NST(4) x NCHUNK(4) x ST_ROWS(1024)] bf16 where element
[p, s, c, r] = a[s*1024 + r, c*128 + p]. The five per-pair reductions
(x.y1, x.y2, |x|^2, |y1|^2, |y2|^2) become partition-axis sums of
elementwise products:
  - products on DVE (tensor_tensor mult, bf16 2x mode, 4096-elem ops)
    and ACT (Square activation) - no accumulate, so ops are big and the
    per-op overhead that dominates fused accum variants is amortized;
  - the reduction over d runs on the otherwise-idle TensorEngine as a
    ones-vector matmul, accumulating the 4 d-chunks into PSUM; each
    256-row group lands on its own PSUM partition (16 groups total).
Epilogue computes per-pair losses on [16, 256] f32 tiles; host sums.

Engine budget per core: DMA ~35us (12.6 MB bf16 at ~358 GB/s), DVE ~28us,
ACT ~30us, PE ~35us - near-balanced at the bf16 memory roofline.
"""

import numpy as np
from contextlib import ExitStack

import concourse.bass as bass
import concourse.bacc as bacc
import concourse.tile as tile
from concourse import mybir
from concourse.bass_utils import run_bass_kernel_spmd

N_CORES = 8
B = 65536
D = 512
PAIRS = B // 2                       # 32768
ROWS_PER_CORE = PAIRS // N_CORES     # 4096
NCHUNK = D // 128                    # 4 d-chunks on partitions
ST_ROWS = 1024                       # rows per supertile
NST = ROWS_PER_CORE // ST_ROWS       # 4 supertiles
GROUP = 256                          # rows per PSUM group (one psum partition)
G_PER_ST = ST_ROWS // GROUP          # 4
NG = NST * G_PER_ST                  # 16 psum partitions used

F32 = mybir.dt.float32
BF16 = mybir.dt.bfloat16
A = mybir.ActivationFunctionType
ALU = mybir.AluOpType


def _body(ctx, tc, out_ap, x_ap, y1_ap, y2_ap):
    nc = tc.nc

    xin = ctx.enter_context(tc.tile_pool(name="xin", bufs=2))
    y1in = ctx.enter_context(tc.tile_pool(name="y1in", bufs=2))
    y2in = ctx.enter_context(tc.tile_pool(name="y2in", bufs=2))
    prods = [ctx.enter_context(tc.tile_pool(name=f"pr{i}", bufs=2))
             for i in range(5)]
    singles = ctx.enter_context(tc.tile_pool(name="singles", bufs=1))
    psum = ctx.enter_context(tc.tile_pool(name="psum", bufs=1, space="PSUM"))
    epi = ctx.enter_context(tc.tile_pool(name="epi", bufs=1))

    # Matmul outputs must start at PSUM partition 0, so group g's sums are
    # routed to partition g via an indicator stationary: gw_g is [128, NG]
    # all-zero except column g (all ones). Every matmul then writes the
    # whole [NG, GROUP] region, accumulating zeros outside group g.
    gws = singles.tile([128, NG * NG], BF16, tag="gws")
    nc.vector.memset(gws[:], 0.0)
    for g in range(NG):
        nc.vector.memset(gws[:, g * NG + g:g * NG + g + 1], 1.0)

    # Warm the ACT table sets during the first DMA (Square now, Ln/Exp for
    # the epilogue) so no table load sits on the critical path later.
    warm = singles.tile([128, 1], F32, tag="warm")
    nc.vector.memset(warm[:], 1.0)
    wo = singles.tile([128, 1], F32, tag="warmout")
    nc.scalar.activation(out=wo[:], in_=warm[:], func=A.Square)
    nc.scalar.activation(out=wo[:], in_=warm[:], func=A.Ln)
    nc.scalar.activation(out=wo[:], in_=warm[:], func=A.Exp)

    # stats[g, stream, :] accumulates group g's per-row sums for stream
    # s in {x.y1, x.y2, x^2, y1^2, y2^2}. Stream stride padded to a full
    # 2 KiB PSUM bank so each matmul output stays inside one bank.
    BANK_F32 = 512
    stats = psum.tile([128, 5, BANK_F32], F32, tag="stats")

    STF = NCHUNK * ST_ROWS           # free elems per supertile (4096)

    for s in range(NST):
        lo, hi = s * STF, (s + 1) * STF
        xt = xin.tile([128, STF], BF16, tag="xt", name="xt")
        nc.sync.dma_start(out=xt[:], in_=x_ap[:, lo:hi])
        y1t = y1in.tile([128, STF], BF16, tag="y1t", name="y1t")
        nc.sync.dma_start(out=y1t[:], in_=y1_ap[:, lo:hi])
        y2t = y2in.tile([128, STF], BF16, tag="y2t", name="y2t")
        nc.sync.dma_start(out=y2t[:], in_=y2_ap[:, lo:hi])

        # Elementwise products, one big op per stream per supertile.
        px = prods[2].tile([128, STF], BF16, tag="px", name="px")
        nc.vector.tensor_tensor(out=px[:], in0=xt[:], in1=xt[:], op=ALU.mult)
        p1 = prods[0].tile([128, STF], BF16, tag="p1", name="p1")
        nc.vector.tensor_tensor(out=p1[:], in0=xt[:], in1=y1t[:], op=ALU.mult)
        p2 = prods[1].tile([128, STF], BF16, tag="p2", name="p2")
        nc.vector.tensor_tensor(out=p2[:], in0=xt[:], in1=y2t[:], op=ALU.mult)
        py1 = prods[3].tile([128, STF], BF16, tag="py1", name="py1")
        nc.scalar.activation(out=py1[:], in_=y1t[:], func=A.Square)
        py2 = prods[4].tile([128, STF], BF16, tag="py2", name="py2")
        nc.scalar.activation(out=py2[:], in_=y2t[:], func=A.Square)

        # Partition-axis reduce on the TensorEngine: indicator^T @ prod
        # chunk. All 4 d-chunks x 16 groups accumulate into one PSUM
        # region per stream across the whole kernel.
        for g in range(G_PER_ST):
            sg = s * G_PER_ST + g
            for sidx, pt in enumerate((p1, p2, px, py1, py2)):
                for c in range(NCHUNK):
                    rlo = c * ST_ROWS + g * GROUP
                    nc.tensor.matmul(
                        stats[0:NG, sidx, 0:GROUP],
                        gws[:, sg * NG:(sg + 1) * NG],
                        pt[:, rlo:rlo + GROUP],
                        start=(s == 0 and g == 0 and c == 0),
                        stop=(s == NST - 1 and g == G_PER_ST - 1
                              and c == NCHUNK - 1),
                    )

    # Epilogue on [NG, 256] f32 tiles (one PSUM->SBUF copy, then SBUF math).
    st = epi.tile([128, 5, GROUP], F32, tag="st")
    nc.vector.tensor_copy(st[0:NG], stats[0:NG, :, 0:GROUP])
    d1, d2, sx, sy1, sy2 = (st[0:NG, i, :] for i in range(5))

    def et(name):
        return epi.tile([128, GROUP], F32, tag=name, name=name)[0:NG]

    q1, q2 = et("q1"), et("q2")
    nc.vector.tensor_tensor(out=q1, in0=sx, in1=sy1, op=ALU.mult)
    nc.vector.tensor_tensor(out=q2, in0=sx, in1=sy2, op=ALU.mult)
    # rsqrt(q) = Exp(-0.5 * Ln(q))
    l1, l2 = et("l1"), et("l2")
    nc.scalar.activation(out=l1, in_=q1, func=A.Ln)
    nc.scalar.activation(out=l2, in_=q2, func=A.Ln)
    r1, r2 = et("r1"), et("r2")
    nc.scalar.activation(out=r1, in_=l1, func=A.Exp, scale=-0.5)
    nc.scalar.activation(out=r2, in_=l2, func=A.Exp, scale=-0.5)
    pos, neg = et("pos"), et("neg")
    nc.vector.tensor_tensor(out=pos, in0=d1, in1=r1, op=ALU.mult)
    nc.vector.tensor_tensor(out=neg, in0=d2, in1=r2, op=ALU.mult)
    z = et("z")
    nc.vector.tensor_tensor(out=z, in0=neg, in1=pos, op=ALU.subtract)
    e = et("e")
    nc.scalar.activation(out=e, in_=z, func=A.Exp)
    loss = et("loss")
    nc.scalar.activation(out=loss, in_=e, func=A.Ln, bias=1.0)
    nc.sync.dma_start(out=out_ap, in_=loss)


def _build():
    nc = bacc.Bacc("TRN2", target_bir_lowering=False, debug=False,
                   num_devices=N_CORES)
    F = NST * NCHUNK * ST_ROWS
    x = nc.dram_tensor("x", [128, F], BF16, kind="ExternalInput").ap()
    y1 = nc.dram_tensor("y1", [128, F], BF16, kind="ExternalInput").ap()
    y2 = nc.dram_tensor("y2", [128, F], BF16, kind="ExternalInput").ap()
    out = nc.dram_tensor("out", [NG, GROUP], F32, kind="ExternalOutput").ap()
    with tile.TileContext(nc) as tc:
        with ExitStack() as ctx:
            _body(ctx, tc, out[:], x[:], y1[:], y2[:])
    nc.compile()
    return nc


_NC_CACHE = None


def _get_nc():
    global _NC_CACHE
    if _NC_CACHE is None:
        _NC_CACHE = _build()
    return _NC_CACHE


def _layout(a_rows):
    # [4096, 512] -> [128, NST*NCHUNK*ST_ROWS] bf16 with
    # t[p, s, c, r] = a[s*ST_ROWS + r, c*128 + p]
    import ml_dtypes
    a = a_rows.astype(ml_dtypes.bfloat16)
    t = a.reshape(NST, ST_ROWS, NCHUNK, 128).transpose(3, 0, 2, 1)
    return np.ascontiguousarray(t.reshape(128, NST * NCHUNK * ST_ROWS))


def _in_maps(query, embed):
    x1 = query[0::2]
    e1 = embed[0::2]
    e2 = embed[1::2]
    maps = []
    for c in range(N_CORES):
        sl = slice(c * ROWS_PER_CORE, (c + 1) * ROWS_PER_CORE)
        maps.append({"x": _layout(x1[sl]), "y1": _layout(e1[sl]),
                     "y2": _layout(e2[sl])})
    return maps


def kernel(query, embed, y, _trace=False):
    query = np.asarray(query, dtype=np.float32)
    embed = np.asarray(embed, dtype=np.float32)
    nc = _get_nc()
    res = run_bass_kernel_spmd(nc, _in_maps(query, embed),
                               core_ids=list(range(N_CORES)), trace=_trace)
    total = 0.0
    for c in range(N_CORES):
        total += res.results[c]["out"].astype(np.float64).sum()
    if _trace:
        kernel._last_results = res
    return np.float32(total / PAIRS)


# revision 9
# speedup vs baseline: 1.4907x; 1.1209x over previous
"""Contrastive loss kernel for Trainium2 (8 NeuronCores, data-parallel).

Reference math (per even/odd row pair i):
    x  = query[2i], y1 = embed[2i], y2 = embed[2i+1]
    pos = <x,y1> / (|x||y1|),  neg = <x,y2> / (|x||y2|)
    loss_i = log(1 + exp(neg - pos))        # = -log_softmax([pos,neg])[0]
    output = mean_i(loss_i)                 # scalar f32

query[1::2] and y are unused by the math. Each core processes 4096 pairs.

Layout: d-on-partition (transposed). Per core each tensor is
[128, NST(4) x NCHUNK(4) x ST_ROWS(1024)] bf16 where element
[p, s, c, r] = a[s*1024 + r, c*128 + p]. The five per-pair reductions
(x.y1, x.y2, |x|^2, |y1|^2, |y2|^2) become partition-axis sums of
elementwise products:
  - products on DVE (tensor_tensor mult, bf16 2x mode, 4096-elem ops)
    and ACT (Square activation) - no accumulate, so ops are big and the
    per-op overhead that dominates fused accum variants is amortized;
  - the reduction over d runs on the otherwise-idle TensorEngine as a
    ones-vector matmul, accumulating the 4 d-chunks into PSUM; each
    256-row group lands on its own PSUM partition (16 groups total).
Epilogue computes per-pair losses on [16, 256] f32 tiles; host sums.

Engine budget per core: DMA ~35us (12.6 MB bf16 at ~358 GB/s), DVE ~28us,
ACT ~30us, PE ~35us - near-balanced at the bf16 memory roofline.
"""

import numpy as np
from contextlib import ExitStack

import concourse.bass as bass
import concourse.bacc as bacc
import concourse.tile as tile
from concourse import mybir
from concourse.bass_utils import run_bass_kernel_spmd

N_CORES = 8
B = 65536
D = 512
PAIRS = B // 2                       # 32768
ROWS_PER_CORE = PAIRS // N_CORES     # 4096
NCHUNK = D // 128                    # 4 d-chunks on partitions
ST_ROWS = 512                        # rows per supertile
NST = ROWS_PER_CORE // ST_ROWS       # 4 supertiles
GROUP = 256                          # rows per PSUM group (one psum partition)
G_PER_ST = ST_ROWS // GROUP          # 4
NG = NST * G_PER_ST                  # 16 psum partitions used

F32 = mybir.dt.float32
BF16 = mybir.dt.bfloat16
A = mybir.ActivationFunctionType
ALU = mybir.AluOpType


def _body(ctx, tc, out_ap, x_ap, y1_ap, y2_ap):
    nc = tc.nc

    xin = ctx.enter_context(tc.tile_pool(name="xin", bufs=2))
    y1in = ctx.enter_context(tc.tile_pool(name="y1in", bufs=2))
    y2in = ctx.enter_context(tc.tile_pool(name="y2in", bufs=2))
    prods = [ctx.enter_context(tc.tile_pool(name=f"pr{i}", bufs=2))
             for i in range(5)]
    singles = ctx.enter_context(tc.tile_pool(name="singles", bufs=1))
    psum = ctx.enter_context(tc.tile_pool(name="psum", bufs=1, space="PSUM"))
    epi = ctx.enter_context(tc.tile_pool(name="epi", bufs=1))

    # Matmul outputs must start at PSUM partition 0, so group g's sums are
    # routed to partition g via an indicator stationary: gw_g is [128, NG]
    # all-zero except column g (all ones). Every matmul then writes the
    # whole [NG, GROUP] region, accumulating zeros outside group g.
    gws = singles.tile([128, NG * NG], BF16, tag="gws")
    nc.vector.memset(gws[:], 0.0)
    for g in range(NG):
        nc.vector.memset(gws[:, g * NG + g:g * NG + g + 1], 1.0)

    # Warm the sqrt table set during the first DMA. That set also
    # contains Square, so the stream Squares and the epilogue Sqrt all
    # run with zero further table loads; only the final Softplus
    # switches sets (one ~1.3us load in the tail).
    warm = singles.tile([128, 1], F32, tag="warm")
    nc.vector.memset(warm[:], 1.0)
    wo = singles.tile([128, 1], F32, tag="warmout")
    nc.scalar.activation(out=wo[:], in_=warm[:], func=A.Sqrt)

    # stats[g, stream, :] accumulates group g's per-row sums for stream
    # s in {x.y1, x.y2, x^2, y1^2, y2^2}. Stream stride padded to a full
    # 2 KiB PSUM bank so each matmul output stays inside one bank.
    BANK_F32 = 512
    stats = psum.tile([128, 5, BANK_F32], F32, tag="stats")

    STF = NCHUNK * ST_ROWS           # free elems per supertile (4096)

    for s in range(NST):
        lo, hi = s * STF, (s + 1) * STF
        xt = xin.tile([128, STF], BF16, tag="xt", name="xt")
        nc.sync.dma_start(out=xt[:], in_=x_ap[:, lo:hi])
        y1t = y1in.tile([128, STF], BF16, tag="y1t", name="y1t")
        nc.sync.dma_start(out=y1t[:], in_=y1_ap[:, lo:hi])
        y2t = y2in.tile([128, STF], BF16, tag="y2t", name="y2t")
        nc.sync.dma_start(out=y2t[:], in_=y2_ap[:, lo:hi])

        # Elementwise products, one big op per stream per supertile.
        px = prods[2].tile([128, STF], BF16, tag="px", name="px")
        nc.vector.tensor_tensor(out=px[:], in0=xt[:], in1=xt[:], op=ALU.mult)
        p1 = prods[0].tile([128, STF], BF16, tag="p1", name="p1")
        nc.vector.tensor_tensor(out=p1[:], in0=xt[:], in1=y1t[:], op=ALU.mult)
        p2 = prods[1].tile([128, STF], BF16, tag="p2", name="p2")
        nc.vector.tensor_tensor(out=p2[:], in0=xt[:], in1=y2t[:], op=ALU.mult)
        py1 = prods[3].tile([128, STF], BF16, tag="py1", name="py1")
        nc.scalar.activation(out=py1[:], in_=y1t[:], func=A.Square)
        py2 = prods[4].tile([128, STF], BF16, tag="py2", name="py2")
        nc.scalar.activation(out=py2[:], in_=y2t[:], func=A.Square)

        # Partition-axis reduce on the TensorEngine: indicator^T @ prod
        # chunk. All 4 d-chunks x 16 groups accumulate into one PSUM
        # region per stream across the whole kernel.
        for g in range(G_PER_ST):
            sg = s * G_PER_ST + g
            for sidx, pt in enumerate((px, p1, p2, py1, py2)):
                for c in range(NCHUNK):
                    rlo = c * ST_ROWS + g * GROUP
                    nc.tensor.matmul(
                        stats[0:NG, sidx, 0:GROUP],
                        gws[:, sg * NG:(sg + 1) * NG],
                        pt[:, rlo:rlo + GROUP],
                        start=(s == 0 and g == 0 and c == 0),
                        stop=(s == NST - 1 and g == G_PER_ST - 1
                              and c == NCHUNK - 1),
                    )

    # Epilogue: one PSUM->SBUF copy, then [NG, 256] f32 SBUF math down to
    # the per-pair logit margin z = neg - pos; softplus + mean run on the
    # host (in f64) over the 4096 z values per core. Sqrt shares the
    # warmed table set, so the tail has zero ACT table loads.
    st = epi.tile([128, 5, GROUP], F32, tag="st", name="st")
    nc.vector.tensor_copy(st[0:NG], stats[0:NG, :, 0:GROUP])
    sx, d1, d2, sy1, sy2 = (st[0:NG, i, :] for i in range(5))

    q = epi.tile([128, 2, GROUP], F32, tag="q", name="q")
    nc.vector.tensor_tensor(out=q[0:NG, 0, :], in0=sx, in1=sy1, op=ALU.mult)
    nc.vector.tensor_tensor(out=q[0:NG, 1, :], in0=sx, in1=sy2, op=ALU.mult)
    # rsqrt(q) = sqrt(1/q): fast approx reciprocal on DVE (~51 ULP),
    # then one Sqrt on ACT (Rsqrt itself is blocked for accuracy).
    rq = epi.tile([128, 2, GROUP], F32, tag="rq", name="rq")
    nc.vector.reciprocal_approx_fast(out=rq[0:NG], in_=q[0:NG])
    r = epi.tile([128, 2, GROUP], F32, tag="r", name="r")
    nc.scalar.activation(out=r[0:NG], in_=rq[0:NG], func=A.Sqrt)
    pos = epi.tile([128, GROUP], F32, tag="pos", name="pos")[0:NG]
    neg = epi.tile([128, GROUP], F32, tag="neg", name="neg")[0:NG]
    nc.vector.tensor_tensor(out=pos, in0=d1, in1=r[0:NG, 0, :], op=ALU.mult)
    nc.vector.tensor_tensor(out=neg, in0=d2, in1=r[0:NG, 1, :], op=ALU.mult)
    z = epi.tile([128, GROUP], F32, tag="z", name="z")[0:NG]
    nc.vector.tensor_tensor(out=z, in0=neg, in1=pos, op=ALU.subtract)
    nc.sync.dma_start(out=out_ap, in_=z)


def _build():
    nc = bacc.Bacc("TRN2", target_bir_lowering=False, debug=False,
                   num_devices=N_CORES)
    F = NST * NCHUNK * ST_ROWS
    x = nc.dram_tensor("x", [128, F], BF16, kind="ExternalInput").ap()
    y1 = nc.dram_tensor("y1", [128, F], BF16, kind="ExternalInput").ap()
    y2 = nc.dram_tensor("y2", [128, F], BF16, kind="ExternalInput").ap()
    out = nc.dram_tensor("out", [NG, GROUP], F32, kind="ExternalOutput").ap()
    with tile.TileContext(nc) as tc:
        with ExitStack() as ctx:
            _body(ctx, tc, out[:], x[:], y1[:], y2[:])
    nc.compile()
    return nc


_NC_CACHE = None


def _get_nc():
    global _NC_CACHE
    if _NC_CACHE is None:
        _NC_CACHE = _build()
    return _NC_CACHE


def _layout(a_rows):
    # [4096, 512] -> [128, NST*NCHUNK*ST_ROWS] bf16 with
    # t[p, s, c, r] = a[s*ST_ROWS + r, c*128 + p]
    import ml_dtypes
    a = a_rows.astype(ml_dtypes.bfloat16)
    t = a.reshape(NST, ST_ROWS, NCHUNK, 128).transpose(3, 0, 2, 1)
    return np.ascontiguousarray(t.reshape(128, NST * NCHUNK * ST_ROWS))


def _in_maps(query, embed):
    x1 = query[0::2]
    e1 = embed[0::2]
    e2 = embed[1::2]
    maps = []
    for c in range(N_CORES):
        sl = slice(c * ROWS_PER_CORE, (c + 1) * ROWS_PER_CORE)
        maps.append({"x": _layout(x1[sl]), "y1": _layout(e1[sl]),
                     "y2": _layout(e2[sl])})
    return maps


def kernel(query, embed, y, _trace=False):
    query = np.asarray(query, dtype=np.float32)
    embed = np.asarray(embed, dtype=np.float32)
    nc = _get_nc()
    res = run_bass_kernel_spmd(nc, _in_maps(query, embed),
                               core_ids=list(range(N_CORES)), trace=_trace)
    total = 0.0
    for c in range(N_CORES):
        z = res.results[c]["out"].astype(np.float64)
        total += np.logaddexp(0.0, z).sum()
    if _trace:
        kernel._last_results = res
    return np.float32(total / PAIRS)
